# revision 51
# baseline (speedup 1.0000x reference)
"""nn_DCAttention Trainium2 kernel v3: folded projections + early split
collectives.

Sharding:
  Phase A (projections): token-parallel, 8 blocks of 512 tokens. The first
  linear of the Q/K branches is folded host-side into the merged conv
  weights (N_k = (P2 C_k + delta_k1 P1) @ W), so each branch is a single
  3-tap conv over x^T (24 matmuls per 128-row output tile). Sequence-edge
  bias corrections are applied to the first/last token column from
  pre-masked per-core correction columns.
  Collective chain (AllToAll x4, issued as data becomes ready):
    cc_v  (V rows)            after the V matmuls
    cc_kd (K^T + exp(delta))  after the K conv
    cc_qa / cc_qb             Q^T halves (pre-scaled by tau'/8)
  Phase B: per (batch, head) flash attention in (dims, tokens) layout;
  exp(delta) folded into the V/ones columns; software-pipelined AV and
  softmax normalization as in v2.
  A2A #2 re-shards attention output token-parallel; out_proj weights are
  prefetched during Phase A so Phase C only waits on the collective.
Output: yT bf16 (1024, 512) per core; assemble() transposes host-side.
"""
import numpy as np
import concourse.bass as bass
import concourse.tile as tile
import concourse.mybir as mybir
from concourse import bacc

f32 = mybir.dt.float32
f32r = mybir.dt.float32r
bf16 = mybir.dt.bfloat16
AF = mybir.ActivationFunctionType
ALU = mybir.AluOpType

D, H, B, L = 1024, 16, 2, 2048
DK = D // H
NCORES = 8
T = (B * L) // NCORES      # 512
TH = T + 2
KT = D // 128              # 8
GROUPS = [[0, 1, 2, 3, 4, 5, 6, 7]]
KD_ROWS = 130              # 128 K^T + 2 expdelta
GELU_FUNC = AF.Gelu

# bcol column indices
BQ, BK, BO, QLO, QHI, KLO, KHI = range(7)

# Schraudolph exp on DVE for these score groups (offloads the ACT engine,
# Phase B's bottleneck). exp(s) ~= bitcast_bf16(int16(s*128/ln2 + 16250.5));
# scores are in [-2, 2] so the int16 affine never leaves [14k, 18.2k].
DVE_EXP_GROUPS = set()      # HW-measured: the int16 DVE path is slower; off
SCH_A = 128.0 / float(np.log(2.0))
SCH_B = 127.0 * 128.0 - 5.5


def build(debug_outputs=(), repeat=1):
    nc = bacc.Bacc(None, target_bir_lowering=False, debug=False)
    nc.num_devices = NCORES

    dp = lambda name, shape, dtype: nc.declare_dram_parameter(name, list(shape), dtype, isOutput=False)
    xr_d = dp("xr", (128, KT, TH), bf16)
    wv_d = dp("wv", (2, 128, KT, 512), bf16)
    cq_d = dp("cq", (KT, 128, 24, 128), bf16)
    ck_d = dp("ck", (KT, 128, 24, 128), bf16)
    owa_d = dp("owa", (KT, 128, 4, 128), bf16)   # out_proj, hh=0 dims (permuted)
    owb_d = dp("owb", (KT, 128, 4, 128), bf16)   # out_proj, hh=1 dims
    bcol_d = dp("bcol", (128, KT, 7), f32)
    bbrd_d = dp("bbrd", (1, D), bf16)          # wv_b
    tau1p_d = dp("tau1p", (128, 16, 4), f32)
    del1p_d = dp("del1p", (128, 16, 4), f32)
    t2w_d = dp("t2w", (128, 16, 16), bf16)
    d2w_d = dp("d2w", (128, 16, 16), bf16)
    t2b_d = dp("t2b", (16, 1), f32)
    d2b_d = dp("d2b", (16, 1), f32)

    yT = nc.declare_dram_parameter("yT", [D, T], bf16, isOutput=True)

    dbg = {}
    for name, shape in [
        ("k3T", (D, T)), ("kT_", (D, T)), ("q3T", (D, T)), ("qT_", (D, T)),
        ("V_", (T, D)), ("taus", (H, T)), ("expd", (H, T)),
        ("kd_out", (NCORES, KD_ROWS, T)), ("attnT", (D, T)),
    ]:
        if name in debug_outputs:
            dbg[name] = nc.declare_dram_parameter("dbg_" + name, list(shape), f32, isOutput=True)

    cc_v_in = nc.dram_tensor("cc_v_in", [NCORES, 128, T], bf16)
    cc_v_out = nc.dram_tensor("cc_v_out", [NCORES, 128, T], bf16)
    cc_kd_in = nc.dram_tensor("cc_kd_in", [NCORES, KD_ROWS, T], bf16)
    cc_kd_out = nc.dram_tensor("cc_kd_out", [NCORES, KD_ROWS, T], bf16)
    cc_q_in = nc.dram_tensor("cc_q_in", [2, NCORES, 64, T], bf16)
    cc_qa_out = nc.dram_tensor("cc_qa_out", [NCORES, 64, T], bf16)
    cc_qb_out = nc.dram_tensor("cc_qb_out", [NCORES, 64, T], bf16)
    a2a2a_in = nc.dram_tensor("a2a2a_in", [NCORES, 64, T], bf16)
    a2a2a_out = nc.dram_tensor("a2a2a_out", [NCORES, 64, T], bf16)
    a2a2b_in = nc.dram_tensor("a2a2b_in", [NCORES, 64, T], bf16)
    a2a2b_out = nc.dram_tensor("a2a2b_out", [NCORES, 64, T], bf16)
    taud = nc.dram_tensor("taud", [H, T], bf16)

    env = dict(locals())
    with tile.TileContext(nc) as tc:
        for _rep in range(repeat):
            _body(nc, tc, env)
    nc.finalize()
    return nc, dbg


def _dbg_copy(nc, pool, dst_ap, src_ap, shape):
    t = pool.tile(list(shape), f32, tag="dbgcp")
    nc.vector.tensor_copy(out=t[:], in_=src_ap)
    nc.sync.dma_start(out=dst_ap, in_=t[:])


def _body(nc, tc, env):
    g = lambda n: env[n]
    dbg = g("dbg")
    cc_v_in, cc_v_out = g("cc_v_in"), g("cc_v_out")
    cc_kd_in, cc_kd_out = g("cc_kd_in"), g("cc_kd_out")
    cc_q_in = g("cc_q_in")
    cc_qa_out, cc_qb_out = g("cc_qa_out"), g("cc_qb_out")
    a2a2a_in, a2a2a_out = g("a2a2a_in"), g("a2a2a_out")
    a2a2b_in, a2a2b_out = g("a2a2b_in"), g("a2a2b_out")
    yT, taud = g("yT"), g("taud")

    # persistent across phases: prefetched out_proj weights, out bias column,
    # and the half-contraction partial of Phase C
    persist_cm = tc.tile_pool(name="persist", bufs=1)
    persist = persist_cm.__enter__()
    owa = persist.tile([128, KT, 4, 128], bf16, tag="owa")
    owb = persist.tile([128, KT, 4, 128], bf16, tag="owb")
    ob = persist.tile([128, KT], f32, tag="ob")
    pc_a = persist.tile([128, KT, T], bf16, tag="pc_a")

    with (
        tc.tile_pool(name="xp", bufs=1) as xp,
        tc.tile_pool(name="const", bufs=1) as constp,
        tc.tile_pool(name="cwpool", bufs=3) as cwpool,    # conv slices
        tc.tile_pool(name="vwpool", bufs=2) as vwpool,
        tc.tile_pool(name="evp", bufs=6) as evp,
        tc.tile_pool(name="taup", bufs=6) as taup,
        tc.tile_pool(name="tdp", bufs=4) as tdp,
        tc.tile_pool(name="gactp", bufs=16) as gactp,
        tc.tile_pool(name="dbgp", bufs=2) as dbgp,
        tc.tile_pool(name="ps", bufs=6, space="PSUM") as ps,
        tc.tile_pool(name="ps_td", bufs=2, space="PSUM") as ps_td,
    ):
        # ---- x^T (bf16, host-pretiled) ----
        xr = xp.tile([128, KT, TH], bf16, tag="xr")
        nc.sync.dma_start(out=xr[:, 0:4, :], in_=g("xr_d")[:, 0:4, :])
        nc.sync.dma_start(out=xr[:, 4:KT, :], in_=g("xr_d")[:, 4:KT, :])

        # ---- constants ----
        bcol = constp.tile([128, KT, 7], f32, tag="bcol")
        nc.sync.dma_start(out=bcol[:], in_=g("bcol_d")[:])
        bvb = constp.tile([128, D], bf16, tag="bvb")
        nc.sync.dma_start(out=bvb[:], in_=g("bbrd_d")[0].unsqueeze(0).broadcast_to([128, D]))
        t2wt = constp.tile([128, 16, 16], bf16, tag="t2wt")
        nc.sync.dma_start(out=t2wt[:], in_=g("t2w_d")[:])
        d2wt = constp.tile([128, 16, 16], bf16, tag="d2wt")
        nc.sync.dma_start(out=d2wt[:], in_=g("d2w_d")[:])
        b_tau2 = constp.tile([16, 1], f32, tag="b_tau2")
        nc.sync.dma_start(out=b_tau2[:], in_=g("t2b_d")[:])
        b_del2 = constp.tile([16, 1], f32, tag="b_del2")
        nc.sync.dma_start(out=b_del2[:], in_=g("d2b_d")[:])
        p1t = constp.tile([128, 16, 4], f32, tag="p1t")
        nc.sync.dma_start(out=p1t[:], in_=g("tau1p_d")[:])
        p1d = constp.tile([128, 16, 4], f32, tag="p1d")
        nc.sync.dma_start(out=p1d[:], in_=g("del1p_d")[:])
        xtd = constp.tile([128, 16, TH], bf16, tag="xtd")

        def branch(c_d, bj, jlo, jhi, pref, q_tau=False, hooks=None, defer=0):
            """Folded 3-tap conv over xr -> proj^T tiles, written to cc.

            hooks[mt] emits extra instructions (small PE contractions) after
            tile mt's matmuls; the first `defer` tiles postpone their
            bias/tau epilogue until after hooks fire (Q waits on taud)."""
            hooks = hooks or {}
            pending = []

            def finish(mt, p):
                ev = evp.tile([128, T], bf16, tag="ev")
                if not q_tau:
                    # bias-add epilogue on the (idle) ACT engine
                    nc.scalar.add(ev[:], p[:], bcol[:, mt, bj:bj + 1])
                    nc.sync.dma_start(out=cc_kd_in[mt, 0:128, :], in_=ev[:])
                else:
                    tb = taup.tile([128, T], bf16, tag="taub")
                    nc.sync.dma_start(out=tb[0:64, :],
                                      in_=taud[2 * mt].unsqueeze(0).broadcast_to([64, T]))
                    nc.sync.dma_start(out=tb[64:128, :],
                                      in_=taud[2 * mt + 1].unsqueeze(0).broadcast_to([64, T]))
                    tmp = taup.tile([128, T], bf16, tag="qtmp")
                    nc.scalar.add(tmp[:], p[:], bcol[:, mt, bj:bj + 1])
                    nc.vector.tensor_tensor(ev[:], tmp[:], tb[:], ALU.mult)
                    nc.sync.dma_start(out=cc_q_in[0, mt], in_=ev[0:64, :])
                    nc.sync.dma_start(out=cc_q_in[1, mt], in_=ev[64:128, :])
                if pref + "T_" in dbg:
                    _dbg_copy(nc, dbgp, dbg[pref + "T_"][mt * 128:(mt + 1) * 128, :],
                              ev[:], (128, T))

            for mt in range(KT):
                cwt = cwpool.tile([128, 24, 128], bf16, tag="c_w")
                nc.sync.dma_start(out=cwt[:, 0:12, :], in_=c_d[mt, :, 0:12, :])
                nc.sync.dma_start(out=cwt[:, 12:24, :], in_=c_d[mt, :, 12:24, :])
                p = ps.tile([128, 512], f32, tag="pA")
                for j in range(24):
                    tap, kt = j // KT, j % KT
                    nc.tensor.matmul(p[:], cwt[:, j, :], xr[:, kt, tap:tap + T],
                                     start=(j == 0), stop=(j == 23))
                # sequence-edge bias corrections (pre-masked, pre-negated)
                nc.vector.tensor_scalar_add(p[:, 0:1], p[:, 0:1],
                                            bcol[:, mt, jlo:jlo + 1])
                nc.vector.tensor_scalar_add(p[:, T - 1:T], p[:, T - 1:T],
                                            bcol[:, mt, jhi:jhi + 1])
                if mt in hooks:
                    hooks[mt]()
                if mt < defer:
                    pending.append((mt, p))
                else:
                    for m2, p2 in pending:
                        finish(m2, p2)
                    pending = []
                    finish(mt, p)

        def td_stage1(p1, tag):
            """tau/delta depthwise conv -> gelu (DVE/ACT only, no PE)."""
            gacts = []
            for gi in range(16):
                xd = xtd[:, gi, :]
                # three taps via 4x-mode tensor_scalar + 2x-mode adds
                # (scalar_tensor_tensor never qualifies for DVE perf modes)
                t0 = tdp.tile([128, T], bf16, tag="t0")
                nc.vector.tensor_scalar(t0[:], xd[:, 0:T], p1[:, gi, 0:1], None, op0=ALU.mult)
                t1 = tdp.tile([128, T], bf16, tag="t1")
                nc.vector.tensor_scalar(t1[:], xd[:, 1:1 + T], p1[:, gi, 1:2], None, op0=ALU.mult)
                t2 = tdp.tile([128, T], bf16, tag="t2")
                nc.vector.tensor_scalar(t2[:], xd[:, 2:2 + T], p1[:, gi, 2:3], None, op0=ALU.mult)
                s01 = tdp.tile([128, T], bf16, tag="s01")
                nc.vector.tensor_tensor(s01[:], t0[:], t1[:], ALU.add)
                mid = tdp.tile([128, T], bf16, tag="mid")
                nc.vector.tensor_tensor(mid[:], s01[:], t2[:], ALU.add)
                gact = gactp.tile([128, T], bf16, tag="gact")
                nc.scalar.activation(out=gact[:], in_=mid[:], func=GELU_FUNC,
                                     bias=p1[:, gi, 3:4], scale=1.0)
                gacts.append(gact)
            return gacts

        def td_stage2(gacts, w2t, bias_t, post, dbg_name):
            """pointwise contraction -> sigmoid [-> post]."""
            pacc = ps_td.tile([16, 512], f32, tag="ptd")
            for gi in range(16):
                nc.tensor.matmul(pacc[:], w2t[:, gi, :], gacts[gi][:],
                                 start=(gi == 0), stop=(gi == 15))
            row = tdp.tile([16, T], f32, tag="td_row")
            nc.scalar.activation(out=row[:], in_=pacc[:], func=AF.Sigmoid, bias=bias_t[:, 0:1])
            out = tdp.tile([16, T], bf16, tag="td_" + dbg_name)
            post(row, out)
            if dbg_name in dbg:
                _dbg_copy(nc, dbgp, dbg[dbg_name][:], out[:], (16, T))
            return out

        # ---- V -> cc_v ----
        for nchunk in range(2):
            vwt = vwpool.tile([128, KT, 512], bf16, tag="v_w")
            nc.sync.dma_start(out=vwt[:, 0:4, :], in_=g("wv_d")[nchunk, :, 0:4, :])
            nc.sync.dma_start(out=vwt[:, 4:KT, :], in_=g("wv_d")[nchunk, :, 4:KT, :])
            for tt in range(4):
                p = ps.tile([128, 512], f32, tag="pA")
                for kt in range(KT):
                    nc.tensor.matmul(p[:], xr[:, kt, 1 + tt * 128:1 + tt * 128 + 128],
                                     vwt[:, kt, :], start=(kt == 0), stop=(kt == KT - 1))
                ev = evp.tile([128, 512], bf16, tag="ev")
                nc.vector.tensor_tensor(ev[:], p[:], bvb[:, nchunk * 512:nchunk * 512 + 512],
                                        ALU.add)
                # one SWDGE DMA scatters this token block into all 4 dest
                # chunks (keeps HWDGE free for the conv weight streams)
                vout = cc_v_in[nchunk * 4:nchunk * 4 + 4].rearrange(
                    "h r t -> h (r t)").rearrange("h (a p d) -> a p h d", p=128, d=128)
                nc.gpsimd.dma_start(out=vout[tt],
                                    in_=ev.rearrange("p (h d) -> p h d", d=128))
                if "V_" in dbg:
                    _dbg_copy(nc, dbgp,
                              dbg["V_"][tt * 128:(tt + 1) * 128,
                                        nchunk * 512:(nchunk + 1) * 512],
                              ev[:], (128, 512))

        nc.gpsimd.collective_compute("AllToAll", ALU.bypass, replica_groups=GROUPS,
                                     ins=[cc_v_in[:]], outs=[cc_v_out[:]])

        # tau/delta conv input, pre-broadcast once for both paths (issued
        # after the V-loop DMAs so it doesn't delay them on HWDGE):
        # xtd[p, gi, c] = xr[(gi%2)*64 + p%64, gi//2, c]
        for ph in range(2):
            for par in range(2):
                nc.gpsimd.dma_start(out=xtd[ph * 64:ph * 64 + 64, par::2, :],
                                    in_=xr[par * 64:par * 64 + 64, :, :])

        # ---- delta gelu stage (DVE/ACT, overlaps K branch PE); K branch
        #      with the small delta contraction hooked near its tail ----
        del_gacts = td_stage1(p1d, "d")

        def emit_expd():
            def post_delta(row, out):
                nc.scalar.activation(out=out[:], in_=row[:], func=AF.Exp,
                                     bias=0.0, scale=1.0)
            expd_row = td_stage2(del_gacts, d2wt, b_del2, post_delta, "expd")
            nc.sync.dma_start(out=cc_kd_in[:, 128:130, :], in_=expd_row[:])

        branch(g("ck_d"), BK, KLO, KHI, "k", hooks={6: emit_expd})

        nc.gpsimd.collective_compute("AllToAll", ALU.bypass, replica_groups=GROUPS,
                                     ins=[cc_kd_in[:]], outs=[cc_kd_out[:]])

        # ---- tau gelu stage; Q branch with tau contraction hooked in ----
        tau_gacts = td_stage1(p1t, "t")

        def emit_tau():
            def post_tau(row, out):
                nc.vector.tensor_scalar(out[:], row[:], 0.125, None, op0=ALU.mult)
            tau_row = td_stage2(tau_gacts, t2wt, b_tau2, post_tau, "taus")
            nc.sync.dma_start(out=taud[:], in_=tau_row[:])

        branch(g("cq_d"), BQ, QLO, QHI, "q", q_tau=True,
               hooks={3: emit_tau}, defer=3)

        # ---- out_proj weight prefetch (DMA idle during Phase B) ----
        nc.sync.dma_start(out=owa[:], in_=g("owa_d").rearrange("m p s j -> p m s j"))
        nc.sync.dma_start(out=owb[:], in_=g("owb_d").rearrange("m p s j -> p m s j"))
        nc.sync.dma_start(out=ob[:], in_=g("bcol_d")[:, :, BO])

    nc.gpsimd.collective_compute("AllToAll", ALU.bypass, replica_groups=GROUPS,
                                 ins=[cc_q_in[0]], outs=[cc_qa_out[:]])
    nc.gpsimd.collective_compute("AllToAll", ALU.bypass, replica_groups=GROUPS,
                                 ins=[cc_q_in[1]], outs=[cc_qb_out[:]])
    if "kd_out" in dbg:
        with tc.tile_pool(name="dk", bufs=2) as dk:
            for s in range(NCORES):
                t_ = dk.tile([KD_ROWS, T], f32, tag="dkc")
                tb_ = dk.tile([KD_ROWS, T], bf16, tag="dkb")
                nc.sync.dma_start(out=tb_[:], in_=cc_kd_out[s])
                nc.vector.tensor_copy(out=t_[:], in_=tb_[:])
                nc.sync.dma_start(out=dbg["kd_out"][s], in_=t_[:])

    # ---- Phase B ----
    with (
        tc.tile_pool(name="hconst", bufs=1) as hcp,
        tc.tile_pool(name="hp", bufs=2) as hp_pool,
        tc.tile_pool(name="ep", bufs=3) as ep,
        tc.tile_pool(name="op", bufs=3) as op_pool,
        tc.tile_pool(name="ps_s", bufs=2, space="PSUM") as ps_s,
        tc.tile_pool(name="ps_o", bufs=2, space="PSUM") as ps_o,
        tc.tile_pool(name="ps_m", bufs=2, space="PSUM") as ps_m,
    ):
        ones64f = hcp.tile([1, 64], f32, tag="ones64f")
        nc.vector.memset(ones64f[:], 1.0)
        ones64 = hcp.tile([1, 64], f32r, tag="ones64")
        nc.vector.tensor_copy(out=ones64[:], in_=ones64f[:])

        prev = None   # (po, e2, grp, vts)
        pend = None   # (po, b, hh, qc)

        def flush_av(st):
            po_, e2_, grp_, vts_ = st
            for i2 in range(2):
                kt = grp_ * 2 + i2
                rhs = e2_[:, i2, :]
                if rhs.dtype == mybir.dt.int16:
                    rhs = rhs.bitcast(bf16)
                nc.tensor.matmul(po_[:], vts_[:, kt, :], rhs,
                                 start=(kt == 0), stop=(kt == 15))

        def normalize(st):
            po_, b_, hh_, qc_ = st
            rs = op_pool.tile([1, T], f32r, tag="rs")
            with nc.allow_low_precision(reason="f32r reciprocal for softmax denom"):
                nc.vector.reciprocal(out=rs[:], in_=po_[64:65, :])
            pb2 = ps_m.tile([64, T], f32, tag="pb2")
            nc.tensor.matmul(pb2[:], ones64[:], rs[:], start=True, stop=True)
            rb = op_pool.tile([64, T], bf16, tag="rb")
            nc.vector.tensor_copy(out=rb[:], in_=pb2[:])
            ot = op_pool.tile([64, T], bf16, tag="ot")
            nc.vector.tensor_tensor(ot[:], po_[0:64, :], rb[:], ALU.mult)
            dst = a2a2a_in if hh_ == 0 else a2a2b_in
            nc.sync.dma_start(out=dst[b_ * 4 + qc_], in_=ot[:])

        for pi, (hh, b) in enumerate([(0, 0), (0, 1), (1, 0), (1, 1)]):
            if True:
                qsrc = cc_qa_out if hh == 0 else cc_qb_out
                blk0 = 4 * b
                kts = hp_pool.tile([64, 4, T], bf16, tag="kts")
                nc.sync.dma_start(out=kts[:],
                                  in_=cc_kd_out[blk0:blk0 + 4, hh * 64:hh * 64 + 64, :]
                                  .transpose([1, 0, 2]))
                qts = hp_pool.tile([64, 4, T], bf16, tag="qts")
                nc.sync.dma_start(out=qts[:],
                                  in_=qsrc[blk0:blk0 + 4].transpose([1, 0, 2]))
                vt = hp_pool.tile([128, 16, 65], bf16, tag="vt")
                nc.vector.memset(vt[:, :, 64:65], 1.0)
                for j in range(4):
                    vsec = cc_v_out[blk0 + j].rearrange("r t -> (r t)").rearrange(
                        "(a p d) -> p a d", p=128, d=128)
                    nc.sync.dma_start(out=vt[:, j * 4:(j + 1) * 4, 0:64],
                                      in_=vsec[:, :, hh * 64:hh * 64 + 64])
                delt = hp_pool.tile([128, 16], bf16, tag="delt")
                for j in range(4):
                    nc.sync.dma_start(out=delt[:, j * 4:(j + 1) * 4],
                                      in_=cc_kd_out[blk0 + j, 128 + hh, :]
                                      .rearrange("(a p) -> p a", p=128))
                deltf = hp_pool.tile([128, 16], f32, tag="deltf")
                nc.vector.tensor_copy(out=deltf[:], in_=delt[:])
                vts = hp_pool.tile([128, 16, 65], bf16, tag="vts")
                for kt in range(16):
                    nc.vector.tensor_scalar(vts[:, kt, :], vt[:, kt, :],
                                            deltf[:, kt:kt + 1], None, op0=ALU.mult)
                kflat = kts.rearrange("p a t -> p (a t)")
                for qc in range(4):
                    po = ps_o.tile([65, T], f32, tag="po")
                    for grp in range(8):
                        s2 = ps_s.tile([128, 2, T], f32, tag="s2")
                        for i2 in range(2):
                            kt = grp * 2 + i2
                            nc.tensor.matmul(s2[:, i2, :], kflat[:, kt * 128:(kt + 1) * 128],
                                             qts[:, qc, :], start=True, stop=True)
                        if prev is not None:
                            flush_av(prev)
                            prev = None
                        if grp == 1 and pend is not None:
                            normalize(pend)
                            pend = None
                        if grp in DVE_EXP_GROUPS:
                            e2 = ep.tile([128, 2, T], mybir.dt.int16, tag="e2i")
                            nc.vector.tensor_scalar(e2[:], s2[:], SCH_A, SCH_B,
                                                    op0=ALU.mult, op1=ALU.add)
                        else:
                            e2 = ep.tile([128, 2, T], bf16, tag="e2")
                            nc.scalar.activation(out=e2[:], in_=s2[:], func=AF.Exp,
                                                 bias=0.0, scale=1.0)
                        prev = (po, e2, grp, vts)
                    pend = (po, b, hh, qc)
            if pi == 1:
                # hh=0 results complete: flush the pipeline and ship them
                # while hh=1 is still computing.
                flush_av(prev)
                prev = None
                normalize(pend)
                pend = None
                nc.gpsimd.collective_compute("AllToAll", ALU.bypass,
                                             replica_groups=GROUPS,
                                             ins=[a2a2a_in[:]], outs=[a2a2a_out[:]])
        flush_av(prev)
        prev = None
        normalize(pend)
        pend = None

    # ---- A2A #2b ----
    nc.gpsimd.collective_compute("AllToAll", ALU.bypass, replica_groups=GROUPS,
                                 ins=[a2a2b_in[:]], outs=[a2a2b_out[:]])

    # ---- Phase C: token-parallel out_proj, two half-contractions ----
    # half a (heads 2s) runs while a2a2b is on the wire
    with (
        tc.tile_pool(name="ca", bufs=1) as ca,
        tc.tile_pool(name="cev", bufs=3) as cev,
        tc.tile_pool(name="ps_c", bufs=4, space="PSUM") as ps_c,
    ):
        at_a = ca.tile([128, 4, T], bf16, tag="at_a")
        nc.sync.dma_start(out=at_a[:],
                          in_=a2a2a_out.rearrange("(s2 sp) r t -> (sp r) s2 t", sp=2))
        for mt in range(KT):
            p = ps_c.tile([128, T], f32, tag="pca")
            for s2 in range(4):
                nc.tensor.matmul(p[:], owa[:, mt, s2, :], at_a[:, s2, :],
                                 start=(s2 == 0), stop=(s2 == 3))
            nc.vector.tensor_copy(out=pc_a[:, mt, :], in_=p[:])
        at_b = ca.tile([128, 4, T], bf16, tag="at_b")
        nc.sync.dma_start(out=at_b[:],
                          in_=a2a2b_out.rearrange("(s2 sp) r t -> (sp r) s2 t", sp=2))
        for mt in range(KT):
            p = ps_c.tile([128, T], f32, tag="pcb")
            for s2 in range(4):
                nc.tensor.matmul(p[:], owb[:, mt, s2, :], at_b[:, s2, :],
                                 start=(s2 == 0), stop=(s2 == 3))
            ev = cev.tile([128, T], bf16, tag="cev")
            nc.vector.scalar_tensor_tensor(ev[:], p[:], ob[:, mt:mt + 1],
                                           pc_a[:, mt, :], op0=ALU.add, op1=ALU.add)
            nc.sync.dma_start(out=yT[mt * 128:(mt + 1) * 128, :], in_=ev[:])

    persist_cm.__exit__(None, None, None)


def make_inputs(full):
    """full: dict of original reference inputs -> list of 8 per-core in_maps."""
    import ml_dtypes
    bf = lambda a: np.ascontiguousarray(np.asarray(a, np.float32)).astype(ml_dtypes.bfloat16)
    f = lambda a: np.ascontiguousarray(np.asarray(a, dtype=np.float32))
    x = np.asarray(full["x"], dtype=np.float32)

    def tile_w(WT, nk):  # WT (nk*128, D) -> (KT, 128, nk, 128)
        return np.ascontiguousarray(
            WT.reshape(nk, 128, KT, 128).transpose(2, 1, 0, 3))

    def conv_w(w):       # (D out, D in, 3) -> (KT mt, 128 p, 24 (tap,kt), 128 m)
        wt = np.asarray(w, np.float32).transpose(2, 1, 0)      # (tap, in, out)
        wt = wt.reshape(3, KT, 128, KT, 128)                   # tap, kt, p, mt, m
        return np.ascontiguousarray(wt.transpose(3, 2, 0, 1, 4).reshape(KT, 128, 24, 128))

    def folded_conv(conv_key, conv_b_key, proj_key, proj_b_key, w_key, wb_key):
        """M_k = P2 C_k (+P1 at k=1); fold the first linear: N_k = M_k W.
        Returns tiled N, bias b3, and edge corrections c_lo/c_hi."""
        C = np.asarray(full[conv_key], np.float32)              # (D, D, 3)
        P = np.asarray(full[proj_key], np.float32)              # (D, 2D)
        W = np.asarray(full[w_key], np.float32)                 # (D, D) torch (out,in)
        wb = f(full[wb_key])
        P1, P2 = P[:, :D], P[:, D:]
        M = np.stack([P2 @ C[:, :, k] for k in range(3)], axis=2)
        M[:, :, 1] += P1
        b2 = f(full[proj_b_key]) + P2 @ f(full[conv_b_key])
        N = np.stack([M[:, :, k] @ W for k in range(3)], axis=2)
        b3 = b2 + (M[:, :, 0] + M[:, :, 1] + M[:, :, 2]) @ wb
        c_lo = M[:, :, 0] @ wb
        c_hi = M[:, :, 2] @ wb
        return conv_w(N), b3, c_lo, c_hi

    nq, bq3, qlo, qhi = folded_conv("convq_w", "convq_b", "qproj_w", "qproj_b",
                                    "Wq_w", "Wq_b")
    nk, bk3, klo, khi = folded_conv("convk_w", "convk_b", "kproj_w", "kproj_b",
                                    "Wk_w", "Wk_b")

    wv = np.asarray(full["Wv_w"], np.float32).T                # (in, out)
    wv_t = np.ascontiguousarray(
        wv.reshape(KT, 128, 2, 512).transpose(2, 1, 0, 3))     # (2, 128, KT, 512)

    bbrd = f(full["Wv_b"]).reshape(1, D)

    perm = np.concatenate([gr * 128 + np.concatenate([np.arange(0, 128, 2),
                                                      np.arange(1, 128, 2)])
                           for gr in range(16)])
    tau1p = np.concatenate([np.asarray(full["tau1_w"], np.float32)[:, 0, :],
                            np.asarray(full["tau1_b"], np.float32)[:, None]], axis=1)[perm]
    del1p = np.concatenate([np.asarray(full["del1_w"], np.float32)[:, 0, :],
                            np.asarray(full["del1_b"], np.float32)[:, None]], axis=1)[perm]
    t2w = np.asarray(full["tau2_w"], np.float32)[:, :, 0].T[perm]  # (2048, 16)
    d2w = np.asarray(full["del2_w"], np.float32)[:, :, 0].T[perm]
    arr3 = lambda a: np.ascontiguousarray(a.reshape(16, 128, a.shape[-1]).transpose(1, 0, 2))

    col = lambda v: np.asarray(v, np.float32).reshape(KT, 128).T  # (128, KT)

    # out_proj split into head-halves, contraction rows permuted so two
    # 64-row sources pack one 128-partition tile:
    #   owa[mt][p=sp*64+r][s2][m] = W^T[head(4*s2+2*sp)*64 + r, mt*128+m]
    WT = np.asarray(full["out_w"], np.float32).T.reshape(16, 64, KT, 128)
    idx = 4 * np.arange(4)[:, None] + 2 * np.arange(2)[None, :]   # (s2, sp)
    owa = WT[idx].transpose(3, 1, 2, 0, 4).reshape(KT, 128, 4, 128)
    owb = WT[idx + 1].transpose(3, 1, 2, 0, 4).reshape(KT, 128, 4, 128)

    common = {
        "wv": bf(wv_t), "cq": bf(nq), "ck": bf(nk),
        "owa": bf(np.ascontiguousarray(owa)), "owb": bf(np.ascontiguousarray(owb)),
        "bbrd": bf(bbrd),
        "tau1p": arr3(tau1p), "del1p": arr3(del1p),
        "t2w": bf(arr3(t2w)), "d2w": bf(arr3(d2w)),
        "t2b": f(full["tau2_b"]).reshape(16, 1), "d2b": f(full["del2_b"]).reshape(16, 1),
    }

    ins = []
    for c in range(NCORES):
        b, t0 = c // 4, (c % 4) * T
        xb = np.zeros((TH, D), np.float32)
        lo, hi = max(t0 - 1, 0), min(t0 + T + 1, L)
        xb[lo - (t0 - 1):hi - (t0 - 1)] = x[b, lo:hi]
        xrt = np.ascontiguousarray(xb.T.reshape(KT, 128, TH).transpose(1, 0, 2))
        flo = 1.0 if t0 == 0 else 0.0
        fhi = 1.0 if t0 + T == L else 0.0
        # columns: BQ, BK, BO, QLO, QHI, KLO, KHI (corrections pre-negated)
        bcol = np.stack([col(bq3), col(bk3), col(f(full["out_b"])),
                         col(-flo * qlo), col(-fhi * qhi),
                         col(-flo * klo), col(-fhi * khi)], axis=2)  # (128, KT, 7)
        m = dict(common)
        m["xr"] = bf(xrt)
        m["bcol"] = np.ascontiguousarray(bcol)
        ins.append(m)
    return ins


def assemble(results):
    y = np.empty((B, L, D), np.float32)
    for c in range(NCORES):
        b, t0 = c // 4, (c % 4) * T
        y[b, t0:t0 + T] = np.asarray(results[c]["yT"], dtype=np.float32).T
    return y


def kernel(**inputs):
    """Takes the full unsharded reference inputs, returns the full (B, L, D) output."""
    from concourse.bass_utils import run_bass_kernel_spmd
    nc, _ = build()
    in_maps = make_inputs(inputs)
    res = run_bass_kernel_spmd(nc, in_maps, list(range(NCORES)))
    return assemble(res.results)


# revision 61
# speedup vs baseline: 1.3185x; 1.3185x over previous
"""nn_DCAttention Trainium2 kernel v3: folded projections + early split
collectives.

Sharding:
  Phase A (projections): token-parallel, 8 blocks of 512 tokens. The first
  linear of the Q/K branches is folded host-side into the merged conv
  weights (N_k = (P2 C_k + delta_k1 P1) @ W), so each branch is a single
  3-tap conv over x^T (24 matmuls per 128-row output tile). Sequence-edge
  bias corrections are applied to the first/last token column from
  pre-masked per-core correction columns.
  Collective chain (AllToAll x4, issued as data becomes ready):
    cc_v  (V rows)            after the V matmuls
    cc_kd (K^T + exp(delta))  after the K conv
    cc_qa / cc_qb             Q^T halves (pre-scaled by tau'/8)
  Phase B: per (batch, head) flash attention in (dims, tokens) layout;
  exp(delta) folded into the V/ones columns; software-pipelined AV and
  softmax normalization as in v2.
  A2A #2 re-shards attention output token-parallel; out_proj weights are
  prefetched during Phase A so Phase C only waits on the collective.
Output: yT bf16 (1024, 512) per core; assemble() transposes host-side.
"""
import numpy as np
import concourse.bass as bass
import concourse.tile as tile
import concourse.mybir as mybir
from concourse import bacc

f32 = mybir.dt.float32
f32r = mybir.dt.float32r
bf16 = mybir.dt.bfloat16
AF = mybir.ActivationFunctionType
ALU = mybir.AluOpType

D, H, B, L = 1024, 16, 2, 2048
DK = D // H
NCORES = 8
T = (B * L) // NCORES      # 512
TH = T + 2
KT = D // 128              # 8
GROUPS = [[0, 1, 2, 3, 4, 5, 6, 7]]
KD_ROWS = 130              # 128 K^T + 2 expdelta
GELU_FUNC = AF.Gelu

# bcol column indices
BQ, BK, BO, QLO, QHI, KLO, KHI = range(7)

# Schraudolph exp on DVE for these score groups (offloads the ACT engine,
# Phase B's bottleneck). exp(s) ~= bitcast_bf16(int16(s*128/ln2 + 16250.5));
# scores are in [-2, 2] so the int16 affine never leaves [14k, 18.2k].
DVE_EXP_GROUPS = set()      # HW-measured: the int16 DVE path is slower; off
SCH_A = 128.0 / float(np.log(2.0))
SCH_B = 127.0 * 128.0 - 5.5

PHASES = "ABC"              # dev knob: which phases _body emits


def build(debug_outputs=(), repeat=1):
    nc = bacc.Bacc(None, target_bir_lowering=False, debug=False)
    nc.num_devices = NCORES

    dp = lambda name, shape, dtype: nc.declare_dram_parameter(name, list(shape), dtype, isOutput=False)
    xr_d = dp("xr", (128, KT, TH), bf16)
    wv_d = dp("wv", (2, 128, KT, 512), bf16)
    cq_d = dp("cq", (KT, 128, 24, 128), bf16)
    ck_d = dp("ck", (KT, 128, 24, 128), bf16)
    owa_d = dp("owa", (KT, 128, 4, 128), bf16)   # out_proj, hh=0 dims (permuted)
    owb_d = dp("owb", (KT, 128, 4, 128), bf16)   # out_proj, hh=1 dims
    bcol_d = dp("bcol", (128, KT, 7), f32)
    bbrd_d = dp("bbrd", (1, D), bf16)          # wv_b
    tau1p_d = dp("tau1p", (128, 16, 4), f32)
    del1p_d = dp("del1p", (128, 16, 4), f32)
    t2w_d = dp("t2w", (128, 16, 16), bf16)
    d2w_d = dp("d2w", (128, 16, 16), bf16)
    t2b_d = dp("t2b", (16, 1), f32)
    d2b_d = dp("d2b", (16, 1), f32)

    yT = nc.declare_dram_parameter("yT", [D, T], bf16, isOutput=True)

    dbg = {}
    for name, shape in [
        ("k3T", (D, T)), ("kT_", (D, T)), ("q3T", (D, T)), ("qT_", (D, T)),
        ("V_", (T, D)), ("taus", (H, T)), ("expd", (H, T)),
        ("kd_out", (NCORES, KD_ROWS, T)), ("attnT", (D, T)),
    ]:
        if name in debug_outputs:
            dbg[name] = nc.declare_dram_parameter("dbg_" + name, list(shape), f32, isOutput=True)

    cc_v_in = nc.dram_tensor("cc_v_in", [NCORES, 128, T], bf16)
    cc_v_out = nc.dram_tensor("cc_v_out", [NCORES, 128, T], bf16)
    cc_kd_in = nc.dram_tensor("cc_kd_in", [NCORES, KD_ROWS, T], bf16)
    cc_kd_out = nc.dram_tensor("cc_kd_out", [NCORES, KD_ROWS, T], bf16)
    cc_q_in = nc.dram_tensor("cc_q_in", [2, NCORES, 64, T], bf16)
    cc_qa_out = nc.dram_tensor("cc_qa_out", [NCORES, 64, T], bf16)
    cc_qb_out = nc.dram_tensor("cc_qb_out", [NCORES, 64, T], bf16)
    a2a2a_in = nc.dram_tensor("a2a2a_in", [NCORES, 64, T], bf16)
    a2a2a_out = nc.dram_tensor("a2a2a_out", [NCORES, 64, T], bf16)
    a2a2b_in = nc.dram_tensor("a2a2b_in", [NCORES, 64, T], bf16)
    a2a2b_out = nc.dram_tensor("a2a2b_out", [NCORES, 64, T], bf16)
    taud = nc.dram_tensor("taud", [H, T], bf16)

    env = dict(locals())
    with tile.TileContext(nc) as tc:
        for _rep in range(repeat):
            _body(nc, tc, env)
    nc.finalize()
    return nc, dbg


def _dbg_copy(nc, pool, dst_ap, src_ap, shape):
    t = pool.tile(list(shape), f32, tag="dbgcp")
    nc.vector.tensor_copy(out=t[:], in_=src_ap)
    nc.sync.dma_start(out=dst_ap, in_=t[:])


def _body(nc, tc, env):
    g = lambda n: env[n]
    dbg = g("dbg")
    cc_v_in, cc_v_out = g("cc_v_in"), g("cc_v_out")
    cc_kd_in, cc_kd_out = g("cc_kd_in"), g("cc_kd_out")
    cc_q_in = g("cc_q_in")
    cc_qa_out, cc_qb_out = g("cc_qa_out"), g("cc_qb_out")
    a2a2a_in, a2a2a_out = g("a2a2a_in"), g("a2a2a_out")
    a2a2b_in, a2a2b_out = g("a2a2b_in"), g("a2a2b_out")
    yT, taud = g("yT"), g("taud")

    # persistent across phases: prefetched out_proj weights, out bias column,
    # and the half-contraction partial of Phase C
    persist_cm = tc.tile_pool(name="persist", bufs=1)
    persist = persist_cm.__enter__()
    owa = persist.tile([128, KT, 4, 128], bf16, tag="owa")
    owb = persist.tile([128, KT, 4, 128], bf16, tag="owb")
    ob = persist.tile([128, KT], f32, tag="ob")
    pc_a = persist.tile([128, KT, T], bf16, tag="pc_a")

    with (
        tc.tile_pool(name="xp", bufs=1) as xp,
        tc.tile_pool(name="const", bufs=1) as constp,
        tc.tile_pool(name="cwpool", bufs=3) as cwpool,    # conv slices
        tc.tile_pool(name="vwpool", bufs=2) as vwpool,
        tc.tile_pool(name="evp", bufs=6) as evp,
        tc.tile_pool(name="taup", bufs=6) as taup,
        tc.tile_pool(name="tdp", bufs=4) as tdp,
        tc.tile_pool(name="gactp", bufs=16) as gactp,
        tc.tile_pool(name="dbgp", bufs=2) as dbgp,
        tc.tile_pool(name="ps", bufs=6, space="PSUM") as ps,
        tc.tile_pool(name="ps_td", bufs=2, space="PSUM") as ps_td,
    ):
        # ---- x^T (bf16, host-pretiled) ----
        xr = xp.tile([128, KT, TH], bf16, tag="xr")
        nc.sync.dma_start(out=xr[:, 0:4, :], in_=g("xr_d")[:, 0:4, :])
        nc.sync.dma_start(out=xr[:, 4:KT, :], in_=g("xr_d")[:, 4:KT, :])

        # ---- constants ----
        bcol = constp.tile([128, KT, 7], f32, tag="bcol")
        nc.sync.dma_start(out=bcol[:], in_=g("bcol_d")[:])
        bvb = constp.tile([128, D], bf16, tag="bvb")
        nc.sync.dma_start(out=bvb[:], in_=g("bbrd_d")[0].unsqueeze(0).broadcast_to([128, D]))
        t2wt = constp.tile([128, 16, 16], bf16, tag="t2wt")
        nc.sync.dma_start(out=t2wt[:], in_=g("t2w_d")[:])
        d2wt = constp.tile([128, 16, 16], bf16, tag="d2wt")
        nc.sync.dma_start(out=d2wt[:], in_=g("d2w_d")[:])
        b_tau2 = constp.tile([16, 1], f32, tag="b_tau2")
        nc.sync.dma_start(out=b_tau2[:], in_=g("t2b_d")[:])
        b_del2 = constp.tile([16, 1], f32, tag="b_del2")
        nc.sync.dma_start(out=b_del2[:], in_=g("d2b_d")[:])
        p1t = constp.tile([128, 16, 4], f32, tag="p1t")
        nc.sync.dma_start(out=p1t[:], in_=g("tau1p_d")[:])
        p1d = constp.tile([128, 16, 4], f32, tag="p1d")
        nc.sync.dma_start(out=p1d[:], in_=g("del1p_d")[:])
        xtd = constp.tile([128, 16, TH], bf16, tag="xtd")

        def branch(c_d, bj, jlo, jhi, pref, q_tau=False, hooks=None, defer=0):
            """Folded 3-tap conv over xr -> proj^T tiles, written to cc.

            hooks[mt] emits extra instructions (small PE contractions) after
            tile mt's matmuls; the first `defer` tiles postpone their
            bias/tau epilogue until after hooks fire (Q waits on taud)."""
            hooks = hooks or {}
            pending = []

            def finish(mt, p):
                ev = evp.tile([128, T], bf16, tag="ev")
                if not q_tau:
                    # bias-add epilogue on the (idle) ACT engine
                    nc.scalar.add(ev[:], p[:], bcol[:, mt, bj:bj + 1])
                    nc.sync.dma_start(out=cc_kd_in[mt, 0:128, :], in_=ev[:])
                else:
                    tb = taup.tile([128, T], bf16, tag="taub")
                    nc.sync.dma_start(out=tb[0:64, :],
                                      in_=taud[2 * mt].unsqueeze(0).broadcast_to([64, T]))
                    nc.sync.dma_start(out=tb[64:128, :],
                                      in_=taud[2 * mt + 1].unsqueeze(0).broadcast_to([64, T]))
                    tmp = taup.tile([128, T], bf16, tag="qtmp")
                    nc.scalar.add(tmp[:], p[:], bcol[:, mt, bj:bj + 1])
                    nc.vector.tensor_tensor(ev[:], tmp[:], tb[:], ALU.mult)
                    nc.sync.dma_start(out=cc_q_in[0, mt], in_=ev[0:64, :])
                    nc.sync.dma_start(out=cc_q_in[1, mt], in_=ev[64:128, :])
                if pref + "T_" in dbg:
                    _dbg_copy(nc, dbgp, dbg[pref + "T_"][mt * 128:(mt + 1) * 128, :],
                              ev[:], (128, T))

            for mt in range(KT):
                cwt = cwpool.tile([128, 24, 128], bf16, tag="c_w")
                nc.sync.dma_start(out=cwt[:, 0:12, :], in_=c_d[mt, :, 0:12, :])
                nc.sync.dma_start(out=cwt[:, 12:24, :], in_=c_d[mt, :, 12:24, :])
                p = ps.tile([128, 512], f32, tag="pA")
                for j in range(24):
                    tap, kt = j // KT, j % KT
                    nc.tensor.matmul(p[:], cwt[:, j, :], xr[:, kt, tap:tap + T],
                                     start=(j == 0), stop=(j == 23))
                # sequence-edge bias corrections (pre-masked, pre-negated)
                nc.vector.tensor_scalar_add(p[:, 0:1], p[:, 0:1],
                                            bcol[:, mt, jlo:jlo + 1])
                nc.vector.tensor_scalar_add(p[:, T - 1:T], p[:, T - 1:T],
                                            bcol[:, mt, jhi:jhi + 1])
                if mt in hooks:
                    hooks[mt]()
                if mt < defer:
                    pending.append((mt, p))
                else:
                    for m2, p2 in pending:
                        finish(m2, p2)
                    pending = []
                    finish(mt, p)

        def td_stage1(p1, tag):
            """tau/delta depthwise conv -> gelu (DVE/ACT only, no PE)."""
            gacts = []
            for gi in range(16):
                xd = xtd[:, gi, :]
                # three taps via 4x-mode tensor_scalar + 2x-mode adds
                # (scalar_tensor_tensor never qualifies for DVE perf modes)
                t0 = tdp.tile([128, T], bf16, tag="t0")
                nc.vector.tensor_scalar(t0[:], xd[:, 0:T], p1[:, gi, 0:1], None, op0=ALU.mult)
                t1 = tdp.tile([128, T], bf16, tag="t1")
                nc.vector.tensor_scalar(t1[:], xd[:, 1:1 + T], p1[:, gi, 1:2], None, op0=ALU.mult)
                t2 = tdp.tile([128, T], bf16, tag="t2")
                nc.vector.tensor_scalar(t2[:], xd[:, 2:2 + T], p1[:, gi, 2:3], None, op0=ALU.mult)
                s01 = tdp.tile([128, T], bf16, tag="s01")
                nc.vector.tensor_tensor(s01[:], t0[:], t1[:], ALU.add)
                mid = tdp.tile([128, T], bf16, tag="mid")
                nc.vector.tensor_tensor(mid[:], s01[:], t2[:], ALU.add)
                gact = gactp.tile([128, T], bf16, tag="gact")
                nc.scalar.activation(out=gact[:], in_=mid[:], func=GELU_FUNC,
                                     bias=p1[:, gi, 3:4], scale=1.0)
                gacts.append(gact)
            return gacts

        def td_stage2(gacts, w2t, bias_t, post, dbg_name):
            """pointwise contraction -> sigmoid [-> post]."""
            pacc = ps_td.tile([16, 512], f32, tag="ptd")
            for gi in range(16):
                nc.tensor.matmul(pacc[:], w2t[:, gi, :], gacts[gi][:],
                                 start=(gi == 0), stop=(gi == 15))
            row = tdp.tile([16, T], f32, tag="td_row")
            nc.scalar.activation(out=row[:], in_=pacc[:], func=AF.Sigmoid, bias=bias_t[:, 0:1])
            out = tdp.tile([16, T], bf16, tag="td_" + dbg_name)
            post(row, out)
            if dbg_name in dbg:
                _dbg_copy(nc, dbgp, dbg[dbg_name][:], out[:], (16, T))
            return out

        # ---- V -> cc_v ----
        for nchunk in range(2):
            vwt = vwpool.tile([128, KT, 512], bf16, tag="v_w")
            nc.sync.dma_start(out=vwt[:, 0:4, :], in_=g("wv_d")[nchunk, :, 0:4, :])
            nc.sync.dma_start(out=vwt[:, 4:KT, :], in_=g("wv_d")[nchunk, :, 4:KT, :])
            for tt in range(4):
                p = ps.tile([128, 512], f32, tag="pA")
                for kt in range(KT):
                    nc.tensor.matmul(p[:], xr[:, kt, 1 + tt * 128:1 + tt * 128 + 128],
                                     vwt[:, kt, :], start=(kt == 0), stop=(kt == KT - 1))
                ev = evp.tile([128, 512], bf16, tag="ev")
                nc.vector.tensor_tensor(ev[:], p[:], bvb[:, nchunk * 512:nchunk * 512 + 512],
                                        ALU.add)
                # one SWDGE DMA scatters this token block into all 4 dest
                # chunks (keeps HWDGE free for the conv weight streams)
                vout = cc_v_in[nchunk * 4:nchunk * 4 + 4].rearrange(
                    "h r t -> h (r t)").rearrange("h (a p d) -> a p h d", p=128, d=128)
                nc.gpsimd.dma_start(out=vout[tt],
                                    in_=ev.rearrange("p (h d) -> p h d", d=128))
                if "V_" in dbg:
                    _dbg_copy(nc, dbgp,
                              dbg["V_"][tt * 128:(tt + 1) * 128,
                                        nchunk * 512:(nchunk + 1) * 512],
                              ev[:], (128, 512))

        nc.gpsimd.collective_compute("AllToAll", ALU.bypass, replica_groups=GROUPS,
                                     ins=[cc_v_in[:]], outs=[cc_v_out[:]])

        # tau/delta conv input, pre-broadcast once for both paths (issued
        # after the V-loop DMAs so it doesn't delay them on HWDGE):
        # xtd[p, gi, c] = xr[(gi%2)*64 + p%64, gi//2, c]
        for ph in range(2):
            for par in range(2):
                nc.gpsimd.dma_start(out=xtd[ph * 64:ph * 64 + 64, par::2, :],
                                    in_=xr[par * 64:par * 64 + 64, :, :])

        # ---- delta gelu stage (DVE/ACT, overlaps K branch PE); K branch
        #      with the small delta contraction hooked near its tail ----
        del_gacts = td_stage1(p1d, "d")

        def emit_expd():
            def post_delta(row, out):
                nc.scalar.activation(out=out[:], in_=row[:], func=AF.Exp,
                                     bias=0.0, scale=1.0)
            expd_row = td_stage2(del_gacts, d2wt, b_del2, post_delta, "expd")
            nc.sync.dma_start(out=cc_kd_in[:, 128:130, :], in_=expd_row[:])

        branch(g("ck_d"), BK, KLO, KHI, "k", hooks={6: emit_expd})

        nc.gpsimd.collective_compute("AllToAll", ALU.bypass, replica_groups=GROUPS,
                                     ins=[cc_kd_in[:]], outs=[cc_kd_out[:]])

        # ---- tau gelu stage; Q branch with tau contraction hooked in ----
        tau_gacts = td_stage1(p1t, "t")

        def emit_tau():
            def post_tau(row, out):
                nc.vector.tensor_scalar(out[:], row[:], 0.125, None, op0=ALU.mult)
            tau_row = td_stage2(tau_gacts, t2wt, b_tau2, post_tau, "taus")
            nc.sync.dma_start(out=taud[:], in_=tau_row[:])

        branch(g("cq_d"), BQ, QLO, QHI, "q", q_tau=True,
               hooks={3: emit_tau}, defer=3)

        # ---- out_proj weight prefetch (DMA idle during Phase B) ----
        nc.sync.dma_start(out=owa[:], in_=g("owa_d").rearrange("m p s j -> p m s j"))
        nc.sync.dma_start(out=owb[:], in_=g("owb_d").rearrange("m p s j -> p m s j"))
        nc.sync.dma_start(out=ob[:], in_=g("bcol_d")[:, :, BO])

    nc.gpsimd.collective_compute("AllToAll", ALU.bypass, replica_groups=GROUPS,
                                 ins=[cc_q_in[0]], outs=[cc_qa_out[:]])
    nc.gpsimd.collective_compute("AllToAll", ALU.bypass, replica_groups=GROUPS,
                                 ins=[cc_q_in[1]], outs=[cc_qb_out[:]])
    if "kd_out" in dbg:
        with tc.tile_pool(name="dk", bufs=2) as dk:
            for s in range(NCORES):
                t_ = dk.tile([KD_ROWS, T], f32, tag="dkc")
                tb_ = dk.tile([KD_ROWS, T], bf16, tag="dkb")
                nc.sync.dma_start(out=tb_[:], in_=cc_kd_out[s])
                nc.vector.tensor_copy(out=t_[:], in_=tb_[:])
                nc.sync.dma_start(out=dbg["kd_out"][s], in_=t_[:])

    # ---- Phase B ----
    with (
        tc.tile_pool(name="hconst", bufs=1) as hcp,
        tc.tile_pool(name="hp", bufs=2) as hp_pool,
        tc.tile_pool(name="ep", bufs=4) as ep,
        tc.tile_pool(name="op", bufs=3) as op_pool,
        tc.tile_pool(name="ps_s", bufs=2, space="PSUM") as ps_s,
        tc.tile_pool(name="ps_o", bufs=2, space="PSUM") as ps_o,
        tc.tile_pool(name="ps_m", bufs=2, space="PSUM") as ps_m,
    ):
        ones64f = hcp.tile([1, 64], f32, tag="ones64f")
        nc.vector.memset(ones64f[:], 1.0)
        ones64 = hcp.tile([1, 64], f32r, tag="ones64")
        nc.vector.tensor_copy(out=ones64[:], in_=ones64f[:])

        pend_av = []  # deque of (po, e2, grp, vts); depth 2 so the PE only
                      # consumes exp outputs finished >=2 groups ago (never
                      # stalls on ACT, keeping the PE pstate ramped)
        pend = None   # (po, b, hh, qc)

        def flush_av(st):
            po_, e2_, grp_, vts_ = st
            for i2 in range(2):
                kt = grp_ * 2 + i2
                rhs = e2_[:, i2, :]
                if rhs.dtype == mybir.dt.int16:
                    rhs = rhs.bitcast(bf16)
                nc.tensor.matmul(po_[:], vts_[:, kt, :], rhs,
                                 start=(kt == 0), stop=(kt == 15))

        def normalize(st):
            po_, b_, hh_, qc_ = st
            rs = op_pool.tile([1, T], f32r, tag="rs")
            with nc.allow_low_precision(reason="f32r reciprocal for softmax denom"):
                nc.vector.reciprocal(out=rs[:], in_=po_[64:65, :])
            pb2 = ps_m.tile([64, T], f32, tag="pb2")
            nc.tensor.matmul(pb2[:], ones64[:], rs[:], start=True, stop=True)
            rb = op_pool.tile([64, T], bf16, tag="rb")
            nc.vector.tensor_copy(out=rb[:], in_=pb2[:])
            ot = op_pool.tile([64, T], bf16, tag="ot")
            nc.vector.tensor_tensor(ot[:], po_[0:64, :], rb[:], ALU.mult)
            dst = a2a2a_in if hh_ == 0 else a2a2b_in
            nc.sync.dma_start(out=dst[b_ * 4 + qc_], in_=ot[:])

        for pi, (hh, b) in enumerate([(0, 0), (0, 1), (1, 0), (1, 1)]):
            if True:
                qsrc = cc_qa_out if hh == 0 else cc_qb_out
                blk0 = 4 * b
                kts = hp_pool.tile([64, 4, T], bf16, tag="kts")
                nc.sync.dma_start(out=kts[:],
                                  in_=cc_kd_out[blk0:blk0 + 4, hh * 64:hh * 64 + 64, :]
                                  .transpose([1, 0, 2]))
                qts = hp_pool.tile([64, 4, T], bf16, tag="qts")
                nc.sync.dma_start(out=qts[:],
                                  in_=qsrc[blk0:blk0 + 4].transpose([1, 0, 2]))
                vt = hp_pool.tile([128, 16, 65], bf16, tag="vt")
                nc.vector.memset(vt[:, :, 64:65], 1.0)
                for j in range(4):
                    vsec = cc_v_out[blk0 + j].rearrange("r t -> (r t)").rearrange(
                        "(a p d) -> p a d", p=128, d=128)
                    nc.sync.dma_start(out=vt[:, j * 4:(j + 1) * 4, 0:64],
                                      in_=vsec[:, :, hh * 64:hh * 64 + 64])
                delt = hp_pool.tile([128, 16], bf16, tag="delt")
                for j in range(4):
                    nc.sync.dma_start(out=delt[:, j * 4:(j + 1) * 4],
                                      in_=cc_kd_out[blk0 + j, 128 + hh, :]
                                      .rearrange("(a p) -> p a", p=128))
                deltf = hp_pool.tile([128, 16], f32, tag="deltf")
                nc.vector.tensor_copy(out=deltf[:], in_=delt[:])
                vts = hp_pool.tile([128, 16, 65], bf16, tag="vts")
                for kt in range(16):
                    nc.vector.tensor_scalar(vts[:, kt, :], vt[:, kt, :],
                                            deltf[:, kt:kt + 1], None, op0=ALU.mult)
                kflat = kts.rearrange("p a t -> p (a t)")
                for qc in range(4):
                    po = ps_o.tile([65, T], f32, tag="po")
                    for grp in range(8):
                        s2 = ps_s.tile([128, 2, T], f32, tag="s2")
                        for i2 in range(2):
                            kt = grp * 2 + i2
                            nc.tensor.matmul(s2[:, i2, :], kflat[:, kt * 128:(kt + 1) * 128],
                                             qts[:, qc, :], start=True, stop=True)
                        if len(pend_av) >= 2:
                            flush_av(pend_av.pop(0))
                        if grp == 1 and pend is not None:
                            normalize(pend)
                            pend = None
                        if grp in DVE_EXP_GROUPS:
                            e2 = ep.tile([128, 2, T], mybir.dt.int16, tag="e2i")
                            nc.vector.tensor_scalar(e2[:], s2[:], SCH_A, SCH_B,
                                                    op0=ALU.mult, op1=ALU.add)
                        else:
                            e2 = ep.tile([128, 2, T], bf16, tag="e2")
                            nc.scalar.activation(out=e2[:], in_=s2[:], func=AF.Exp,
                                                 bias=0.0, scale=1.0)
                        pend_av.append((po, e2, grp, vts))
                    pend = (po, b, hh, qc)
            if pi == 1:
                # hh=0 results complete: flush the pipeline and ship them
                # while hh=1 is still computing.
                while pend_av:
                    flush_av(pend_av.pop(0))
                normalize(pend)
                pend = None
                nc.gpsimd.collective_compute("AllToAll", ALU.bypass,
                                             replica_groups=GROUPS,
                                             ins=[a2a2a_in[:]], outs=[a2a2a_out[:]])
        while pend_av:
            flush_av(pend_av.pop(0))
        normalize(pend)
        pend = None

    # ---- A2A #2b ----
    nc.gpsimd.collective_compute("AllToAll", ALU.bypass, replica_groups=GROUPS,
                                 ins=[a2a2b_in[:]], outs=[a2a2b_out[:]])

    # ---- Phase C: token-parallel out_proj, two half-contractions ----
    # half a (heads 2s) runs while a2a2b is on the wire
    with (
        tc.tile_pool(name="ca", bufs=1) as ca,
        tc.tile_pool(name="cev", bufs=3) as cev,
        tc.tile_pool(name="ps_c", bufs=4, space="PSUM") as ps_c,
    ):
        at_a = ca.tile([128, 4, T], bf16, tag="at_a")
        nc.sync.dma_start(out=at_a[:],
                          in_=a2a2a_out.rearrange("(s2 sp) r t -> (sp r) s2 t", sp=2))
        for mt in range(KT):
            p = ps_c.tile([128, T], f32, tag="pca")
            for s2 in range(4):
                nc.tensor.matmul(p[:], owa[:, mt, s2, :], at_a[:, s2, :],
                                 start=(s2 == 0), stop=(s2 == 3))
            nc.vector.tensor_copy(out=pc_a[:, mt, :], in_=p[:])
        at_b = ca.tile([128, 4, T], bf16, tag="at_b")
        nc.sync.dma_start(out=at_b[:],
                          in_=a2a2b_out.rearrange("(s2 sp) r t -> (sp r) s2 t", sp=2))
        for mt in range(KT):
            p = ps_c.tile([128, T], f32, tag="pcb")
            for s2 in range(4):
                nc.tensor.matmul(p[:], owb[:, mt, s2, :], at_b[:, s2, :],
                                 start=(s2 == 0), stop=(s2 == 3))
            ev = cev.tile([128, T], bf16, tag="cev")
            nc.vector.scalar_tensor_tensor(ev[:], p[:], ob[:, mt:mt + 1],
                                           pc_a[:, mt, :], op0=ALU.add, op1=ALU.add)
            nc.sync.dma_start(out=yT[mt * 128:(mt + 1) * 128, :], in_=ev[:])

    persist_cm.__exit__(None, None, None)


def make_inputs(full):
    """full: dict of original reference inputs -> list of 8 per-core in_maps."""
    import ml_dtypes
    bf = lambda a: np.ascontiguousarray(np.asarray(a, np.float32)).astype(ml_dtypes.bfloat16)
    f = lambda a: np.ascontiguousarray(np.asarray(a, dtype=np.float32))
    x = np.asarray(full["x"], dtype=np.float32)

    def tile_w(WT, nk):  # WT (nk*128, D) -> (KT, 128, nk, 128)
        return np.ascontiguousarray(
            WT.reshape(nk, 128, KT, 128).transpose(2, 1, 0, 3))

    def conv_w(w):       # (D out, D in, 3) -> (KT mt, 128 p, 24 (tap,kt), 128 m)
        wt = np.asarray(w, np.float32).transpose(2, 1, 0)      # (tap, in, out)
        wt = wt.reshape(3, KT, 128, KT, 128)                   # tap, kt, p, mt, m
        return np.ascontiguousarray(wt.transpose(3, 2, 0, 1, 4).reshape(KT, 128, 24, 128))

    def folded_conv(conv_key, conv_b_key, proj_key, proj_b_key, w_key, wb_key):
        """M_k = P2 C_k (+P1 at k=1); fold the first linear: N_k = M_k W.
        Returns tiled N, bias b3, and edge corrections c_lo/c_hi."""
        C = np.asarray(full[conv_key], np.float32)              # (D, D, 3)
        P = np.asarray(full[proj_key], np.float32)              # (D, 2D)
        W = np.asarray(full[w_key], np.float32)                 # (D, D) torch (out,in)
        wb = f(full[wb_key])
        P1, P2 = P[:, :D], P[:, D:]
        M = np.stack([P2 @ C[:, :, k] for k in range(3)], axis=2)
        M[:, :, 1] += P1
        b2 = f(full[proj_b_key]) + P2 @ f(full[conv_b_key])
        N = np.stack([M[:, :, k] @ W for k in range(3)], axis=2)
        b3 = b2 + (M[:, :, 0] + M[:, :, 1] + M[:, :, 2]) @ wb
        c_lo = M[:, :, 0] @ wb
        c_hi = M[:, :, 2] @ wb
        return conv_w(N), b3, c_lo, c_hi

    nq, bq3, qlo, qhi = folded_conv("convq_w", "convq_b", "qproj_w", "qproj_b",
                                    "Wq_w", "Wq_b")
    nk, bk3, klo, khi = folded_conv("convk_w", "convk_b", "kproj_w", "kproj_b",
                                    "Wk_w", "Wk_b")

    wv = np.asarray(full["Wv_w"], np.float32).T                # (in, out)
    wv_t = np.ascontiguousarray(
        wv.reshape(KT, 128, 2, 512).transpose(2, 1, 0, 3))     # (2, 128, KT, 512)

    bbrd = f(full["Wv_b"]).reshape(1, D)

    perm = np.concatenate([gr * 128 + np.concatenate([np.arange(0, 128, 2),
                                                      np.arange(1, 128, 2)])
                           for gr in range(16)])
    tau1p = np.concatenate([np.asarray(full["tau1_w"], np.float32)[:, 0, :],
                            np.asarray(full["tau1_b"], np.float32)[:, None]], axis=1)[perm]
    del1p = np.concatenate([np.asarray(full["del1_w"], np.float32)[:, 0, :],
                            np.asarray(full["del1_b"], np.float32)[:, None]], axis=1)[perm]
    t2w = np.asarray(full["tau2_w"], np.float32)[:, :, 0].T[perm]  # (2048, 16)
    d2w = np.asarray(full["del2_w"], np.float32)[:, :, 0].T[perm]
    arr3 = lambda a: np.ascontiguousarray(a.reshape(16, 128, a.shape[-1]).transpose(1, 0, 2))

    col = lambda v: np.asarray(v, np.float32).reshape(KT, 128).T  # (128, KT)

    # out_proj split into head-halves, contraction rows permuted so two
    # 64-row sources pack one 128-partition tile:
    #   owa[mt][p=sp*64+r][s2][m] = W^T[head(4*s2+2*sp)*64 + r, mt*128+m]
    WT = np.asarray(full["out_w"], np.float32).T.reshape(16, 64, KT, 128)
    idx = 4 * np.arange(4)[:, None] + 2 * np.arange(2)[None, :]   # (s2, sp)
    owa = WT[idx].transpose(3, 1, 2, 0, 4).reshape(KT, 128, 4, 128)
    owb = WT[idx + 1].transpose(3, 1, 2, 0, 4).reshape(KT, 128, 4, 128)

    common = {
        "wv": bf(wv_t), "cq": bf(nq), "ck": bf(nk),
        "owa": bf(np.ascontiguousarray(owa)), "owb": bf(np.ascontiguousarray(owb)),
        "bbrd": bf(bbrd),
        "tau1p": arr3(tau1p), "del1p": arr3(del1p),
        "t2w": bf(arr3(t2w)), "d2w": bf(arr3(d2w)),
        "t2b": f(full["tau2_b"]).reshape(16, 1), "d2b": f(full["del2_b"]).reshape(16, 1),
    }

    ins = []
    for c in range(NCORES):
        b, t0 = c // 4, (c % 4) * T
        xb = np.zeros((TH, D), np.float32)
        lo, hi = max(t0 - 1, 0), min(t0 + T + 1, L)
        xb[lo - (t0 - 1):hi - (t0 - 1)] = x[b, lo:hi]
        xrt = np.ascontiguousarray(xb.T.reshape(KT, 128, TH).transpose(1, 0, 2))
        flo = 1.0 if t0 == 0 else 0.0
        fhi = 1.0 if t0 + T == L else 0.0
        # columns: BQ, BK, BO, QLO, QHI, KLO, KHI (corrections pre-negated)
        bcol = np.stack([col(bq3), col(bk3), col(f(full["out_b"])),
                         col(-flo * qlo), col(-fhi * qhi),
                         col(-flo * klo), col(-fhi * khi)], axis=2)  # (128, KT, 7)
        m = dict(common)
        m["xr"] = bf(xrt)
        m["bcol"] = np.ascontiguousarray(bcol)
        ins.append(m)
    return ins


def assemble(results):
    y = np.empty((B, L, D), np.float32)
    for c in range(NCORES):
        b, t0 = c // 4, (c % 4) * T
        y[b, t0:t0 + T] = np.asarray(results[c]["yT"], dtype=np.float32).T
    return y


def kernel(**inputs):
    """Takes the full unsharded reference inputs, returns the full (B, L, D) output."""
    from concourse.bass_utils import run_bass_kernel_spmd
    nc, _ = build()
    in_maps = make_inputs(inputs)
    res = run_bass_kernel_spmd(nc, in_maps, list(range(NCORES)))
    return assemble(res.results)
